# revision 1
# baseline (speedup 1.0000x reference)
"""AttentionPITF Trainium2 kernel — 8-core data-parallel (batch-sharded).

Pipeline (per core, 2048 batch rows, 16 tiles of 128):
  - hist (tagUserVecs[x[:,4:54]]) gathered FEATURE-major in bf16 via
    dma_gather(transpose=True). int16 index limit handled by splitting the
    50000-row table into lo (<32768) / hi halves; out-of-half indices point at
    zero rows and the two gathers are merged with one DVE add.
  - z^T = W_att @ hist^T on PE (bf16), relu+bias on ACT (PSUM evac).
  - scores s[c] = sum_j relu(z)[j,c] * u[b(c),j]: DVE mul with free-dim
    broadcast of u^T, then ones-block matmul on PE -> s broadcast across
    partitions in PSUM.
  - e = exp(s) on ACT (softmax un-normalized; values are tiny, no max-sub).
  - g^T = sum_m hist^T * e (DVE scale + fold + reduce), wsum likewise from e.
  - h^T = g^T / wsum; mix^T = relu(W_map @ [u;h;u-h;u*h]^T) on PE/ACT.
  - r = ones^T @ (mix^T*(ut-nut)^T + iv^T*(it-nit)^T) in fp32 on PE.
"""

import numpy as np
import ml_dtypes

import concourse.bass as bass
import concourse.bacc as bacc
import concourse.mybir as mybir
from concourse.tile import TileContext
from concourse.bass_utils import run_bass_kernel_spmd

BF16 = ml_dtypes.bfloat16

NCORES = 8
B, K, M = 16384, 256, 50
BC = B // NCORES          # 2048 rows per core
P = 128
NT = BC // P              # 16 tiles per core
CPT = M * P               # 6400 gathered columns per tile (m-major: c = m*128 + b)
SPLIT = 32768             # int16-safe split of the 50000-row tag table
NTAG = 50000
NHI = NTAG - SPLIT + 1    # hi half + one zero row (index NHI-1)
CHUNK = 512               # PE moving-operand chunk
NCHUNK = (CPT + CHUNK - 1) // CHUNK   # 13 (12*512 + 256)

_nc_cache = {}


def _build_program(nt=NT, stage=99):
    nc = bacc.Bacc()
    dt = mybir.dt

    # ---- DRAM I/O ----
    tab_lo = nc.dram_tensor("tab_lo", [SPLIT, K], dt.bfloat16, kind="ExternalInput")
    tab_hi = nc.dram_tensor("tab_hi", [NHI, K], dt.bfloat16, kind="ExternalInput")
    idx_lo = nc.dram_tensor("idx_lo", [nt * P, CPT // 16], dt.int16, kind="ExternalInput")
    idx_hi = nc.dram_tensor("idx_hi", [nt * P, CPT // 16], dt.int16, kind="ExternalInput")
    w_attT = nc.dram_tensor("w_attT", [P, 2, K], dt.bfloat16, kind="ExternalInput")   # [p, kt, j]
    w_mapT = nc.dram_tensor("w_mapT", [P, 8, K], dt.bfloat16, kind="ExternalInput")   # [p, kt', j]
    b_att = nc.dram_tensor("b_att", [K, 1], dt.float32, kind="ExternalInput")
    b_map = nc.dram_tensor("b_map", [K, 1], dt.float32, kind="ExternalInput")
    uT_bf = nc.dram_tensor("uT_bf", [P, 2, BC], dt.bfloat16, kind="ExternalInput")
    auxT = nc.dram_tensor("auxT", [P, 2, 5, BC], dt.float32, kind="ExternalInput")
    r_out = nc.dram_tensor("r_out", [nt, P], dt.float32, kind="ExternalOutput")

    with TileContext(nc) as tc:
        with (
            tc.tile_pool(name="const", bufs=1) as cpool,
            tc.tile_pool(name="hist", bufs=2) as hpool,
            tc.tile_pool(name="zbuf", bufs=2) as zpool,
            tc.tile_pool(name="ebuf", bufs=2) as epool,
            tc.tile_pool(name="small", bufs=2) as spool,
            tc.tile_pool(name="idxp", bufs=3) as ipool,
            tc.tile_pool(name="psz", bufs=3, space="PSUM") as psz,
            tc.tile_pool(name="pss", bufs=2, space="PSUM") as pss,
            tc.tile_pool(name="psm", bufs=1, space="PSUM") as psm,
        ):
            # ---- constants ----
            watt_s = cpool.tile([P, 2, K], dt.bfloat16)     # lhsT [k,j]: [:, kt, j]
            nc.sync.dma_start(out=watt_s[:], in_=w_attT[:, :, :])
            wmap_s = cpool.tile([P, 8, K], dt.bfloat16)     # lhsT [k',j]: [:, kt', j]
            nc.sync.dma_start(out=wmap_s[:], in_=w_mapT[:, :, :])
            batt_s = cpool.tile([P, 2], dt.float32)
            nc.sync.dma_start(out=batt_s[:, :], in_=b_att.rearrange("(a p) o -> p (a o)", p=P))
            bmap_s = cpool.tile([P, 2], dt.float32)
            nc.sync.dma_start(out=bmap_s[:, :], in_=b_map.rearrange("(a p) o -> p (a o)", p=P))
            ones_bf = cpool.tile([P, P], dt.bfloat16)
            nc.gpsimd.memset(ones_bf[:], 1.0)
            ones_f32 = cpool.tile([P, 1], dt.float32)
            nc.gpsimd.memset(ones_f32[:], 1.0)

            for t in range(nt):
                bs = t * P  # batch col range [bs, bs+P)

                # ---- index tiles ----
                ixl = ipool.tile([P, CPT // 16], dt.int16, tag="ixl")
                ixh = ipool.tile([P, CPT // 16], dt.int16, tag="ixh")
                nc.scalar.dma_start(out=ixl[:, :], in_=idx_lo[t * P:(t + 1) * P, :])
                nc.scalar.dma_start(out=ixh[:, :], in_=idx_hi[t * P:(t + 1) * P, :])

                # ---- feature-major hist gathers: [128, kt, c] bf16 ----
                hist = hpool.tile([P, 2, CPT], dt.bfloat16, tag="hlo")
                hist_hi = hpool.tile([P, 2, CPT], dt.bfloat16, tag="hhi")
                nc.gpsimd.dma_gather(
                    out_ap=hist[:], in_ap=tab_lo[:], idxs_ap=ixl[:],
                    num_idxs=CPT, num_idxs_reg=CPT, elem_size=K, transpose=True,
                    single_packet=False)
                nc.gpsimd.dma_gather(
                    out_ap=hist_hi[:], in_ap=tab_hi[:], idxs_ap=ixh[:],
                    num_idxs=CPT, num_idxs_reg=CPT, elem_size=K, transpose=True,
                    single_packet=False)
                # merge halves (exactly one is nonzero per column)
                nc.vector.tensor_add(
                    out=hist.rearrange("p a c -> p (a c)"),
                    in0=hist.rearrange("p a c -> p (a c)"),
                    in1=hist_hi.rearrange("p a c -> p (a c)"))

                # ---- per-tile u (feature-major bf16) ----
                u_s = spool.tile([P, 2, P], dt.bfloat16, tag="u")
                nc.sync.dma_start(out=u_s[:], in_=uT_bf[:, :, bs:bs + P])

                # ---- z = W_att @ histT ; relu ; * u ----
                if stage < 2:
                    r_s = spool.tile([P, P], dt.float32, tag="r")
                    nc.vector.tensor_copy(out=r_s[0:1, :], in_=hist[0:1, 0, 0:P])
                    nc.sync.dma_start(out=r_out[t:t + 1, :], in_=r_s[0:1, :])
                    continue
                relu_z = zpool.tile([P, 2, CPT], dt.bfloat16, tag="rz")
                for ch in range(NCHUNK):
                    c0 = ch * CHUNK
                    cw = min(CHUNK, CPT - c0)
                    for jt in range(2):
                        pz = psz.tile([P, CHUNK], dt.float32, tag="pz")
                        for kt in range(2):
                            nc.tensor.matmul(
                                out=pz[:, :cw],
                                lhsT=watt_s[:, kt, jt * P:(jt + 1) * P],
                                rhs=hist[:, kt, c0:c0 + cw],
                                start=(kt == 0), stop=(kt == 1))
                        nc.scalar.activation(
                            out=relu_z[:, jt, c0:c0 + cw], in_=pz[:, :cw],
                            func=mybir.ActivationFunctionType.Relu,
                            bias=batt_s[:, jt:jt + 1])

                if stage < 3:
                    r_s = spool.tile([P, P], dt.float32, tag="r")
                    nc.vector.tensor_copy(out=r_s[0:1, :], in_=relu_z[0:1, 0, 0:P])
                    nc.sync.dma_start(out=r_out[t:t + 1, :], in_=r_s[0:1, :])
                    continue
                # tmp1 = relu_z * uT (broadcast u cols over m) — in place
                for jt in range(2):
                    ub = u_s[:, jt, :].unsqueeze(1).broadcast_to([P, M, P])
                    nc.vector.tensor_mul(
                        out=relu_z[:, jt, :].rearrange("p (m b) -> p m b", b=P),
                        in0=relu_z[:, jt, :].rearrange("p (m b) -> p m b", b=P),
                        in1=ub)

                # ---- scores: s_bc[p, c] = sum_j tmp1[j, c] (broadcast over p) ----
                e_s = epool.tile([P, CPT], dt.bfloat16, tag="e")
                for ch in range(NCHUNK):
                    c0 = ch * CHUNK
                    cw = min(CHUNK, CPT - c0)
                    ps = pss.tile([P, CHUNK], dt.float32, tag="ps")
                    for jt in range(2):
                        nc.tensor.matmul(
                            out=ps[:, :cw], lhsT=ones_bf[:],
                            rhs=relu_z[:, jt, c0:c0 + cw],
                            start=(jt == 0), stop=(jt == 1))
                    nc.scalar.activation(
                        out=e_s[:, c0:c0 + cw], in_=ps[:, :cw],
                        func=mybir.ActivationFunctionType.Exp)

                if stage < 4:
                    r_s = spool.tile([P, P], dt.float32, tag="r")
                    nc.vector.tensor_copy(out=r_s[0:1, :], in_=e_s[0:1, 0:P])
                    nc.sync.dma_start(out=r_out[t:t + 1, :], in_=r_s[0:1, :])
                    continue
                # ---- g^T = sum_m histT * e ; wsum = sum_m e ----
                # scale hist by e (broadcast over kt halves), in place
                for kt in range(2):
                    nc.vector.tensor_mul(out=hist[:, kt, :], in0=hist[:, kt, :], in1=e_s[:])
                # fold m to 1 via strided tree adds (2x bf16 mode)
                hv = hist.rearrange("p a (m b) -> p a m b", b=P)
                ev = e_s.rearrange("p (m b) -> p m b", b=P)
                g_s = spool.tile([P, 2, P], dt.float32, tag="g")
                w_s = spool.tile([P, P], dt.float32, tag="w")
                # levels: 50->25->13->7->4->2->1 (last two levels write f32)
                for n, h in ((50, 25), (25, 13), (13, 7), (7, 4), (4, 2)):
                    k = n - h  # back-half size (>= h)
                    nc.vector.tensor_add(out=hv[:, :, 0:k, :], in0=hv[:, :, 0:k, :],
                                         in1=hv[:, :, h:n, :])
                    nc.vector.tensor_add(out=ev[:, 0:k, :], in0=ev[:, 0:k, :],
                                         in1=ev[:, h:n, :])
                nc.vector.tensor_add(out=g_s[:], in0=hv[:, :, 0, :], in1=hv[:, :, 1, :])
                nc.vector.tensor_add(out=w_s[:], in0=ev[:, 0, :], in1=ev[:, 1, :])

                # ---- h = g / wsum ; mix inputs ----
                inv_s = spool.tile([P, P], dt.float32, tag="inv")
                nc.vector.reciprocal(out=inv_s[:], in_=w_s[:])
                h_s = spool.tile([P, 2, P], dt.bfloat16, tag="h")
                ib = inv_s[:].unsqueeze(1).broadcast_to([P, 2, P])
                nc.vector.tensor_mul(out=h_s[:], in0=g_s[:], in1=ib)
                umh = spool.tile([P, 2, P], dt.bfloat16, tag="umh")
                nc.vector.tensor_sub(out=umh[:], in0=u_s[:], in1=h_s[:])
                uxh = spool.tile([P, 2, P], dt.bfloat16, tag="uxh")
                nc.vector.tensor_mul(out=uxh[:], in0=u_s[:], in1=h_s[:])

                if stage < 5:
                    r_s = spool.tile([P, P], dt.float32, tag="r")
                    nc.vector.tensor_copy(out=r_s[0:1, :], in_=h_s[0:1, 0, :])
                    nc.sync.dma_start(out=r_out[t:t + 1, :], in_=r_s[0:1, :])
                    continue
                # ---- mix^T = relu(W_map @ cat^T + b_map) ----
                cat_srcs = [u_s[:, 0, :], u_s[:, 1, :], h_s[:, 0, :], h_s[:, 1, :],
                            umh[:, 0, :], umh[:, 1, :], uxh[:, 0, :], uxh[:, 1, :]]
                mix_s = spool.tile([P, 2, P], dt.float32, tag="mix")
                for jt in range(2):
                    pm = psm.tile([P, P], dt.float32, tag="pm")
                    for kt in range(8):
                        nc.tensor.matmul(
                            out=pm[:], lhsT=wmap_s[:, kt, jt * P:(jt + 1) * P],
                            rhs=cat_srcs[kt], start=(kt == 0), stop=(kt == 7))
                    nc.scalar.activation(
                        out=mix_s[:, jt, :], in_=pm[:],
                        func=mybir.ActivationFunctionType.Relu,
                        bias=bmap_s[:, jt:jt + 1])

                if stage < 6:
                    r_s = spool.tile([P, P], dt.float32, tag="r")
                    nc.vector.tensor_copy(out=r_s[0:1, :], in_=mix_s[0:1, 0, :])
                    nc.sync.dma_start(out=r_out[t:t + 1, :], in_=r_s[0:1, :])
                    continue
                # ---- final dots (fp32) ----
                aux_s = spool.tile([P, 2, 5, P], dt.float32, tag="aux")
                nc.scalar.dma_start(out=aux_s[:], in_=auxT[:, :, :, bs:bs + P])
                dt_s = aux_s[:, :, 0, :]
                di_s = aux_s[:, :, 2, :]
                nc.vector.tensor_sub(out=dt_s, in0=aux_s[:, :, 0, :], in1=aux_s[:, :, 1, :])
                nc.vector.tensor_sub(out=di_s, in0=aux_s[:, :, 2, :], in1=aux_s[:, :, 3, :])
                nc.vector.tensor_mul(out=dt_s, in0=dt_s, in1=mix_s[:])  # P1
                nc.vector.tensor_mul(out=di_s, in0=di_s, in1=aux_s[:, :, 4, :])   # P2
                pr = psm.tile([P, P], dt.float32, tag="pr")
                srcs = [dt_s[:, 0], dt_s[:, 1], di_s[:, 0], di_s[:, 1]]
                for i, srcap in enumerate(srcs):
                    nc.tensor.matmul(out=pr[0:1, :], lhsT=ones_f32[:], rhs=srcap,
                                     start=(i == 0), stop=(i == 3))
                r_s = spool.tile([P, P], dt.float32, tag="r")
                nc.vector.tensor_copy(out=r_s[0:1, :], in_=pr[0:1, :])
                nc.sync.dma_start(out=r_out[t:t + 1, :], in_=r_s[0:1, :])

    nc.compile()
    return nc


def _host_prep(inputs):
    x = np.asarray(inputs["x"])
    userVecs = np.asarray(inputs["userVecs"], np.float32)
    itemVecs = np.asarray(inputs["itemVecs"], np.float32)
    tagU = np.asarray(inputs["tagUserVecs"], np.float32)
    tagI = np.asarray(inputs["tagItemVecs"], np.float32)
    W_att = np.asarray(inputs["W_att"], np.float32)
    b_att = np.asarray(inputs["b_att"], np.float32)
    W_map = np.asarray(inputs["W_map"], np.float32)
    b_map = np.asarray(inputs["b_map"], np.float32)

    tab_lo = np.ascontiguousarray(tagU[:SPLIT]).astype(BF16)
    tab_hi = np.concatenate([tagU[SPLIT:], np.zeros((1, K), np.float32)], 0).astype(BF16)
    w_attT = np.ascontiguousarray(W_att.T.reshape(2, P, K).transpose(1, 0, 2)).astype(BF16)
    w_mapT = np.ascontiguousarray(W_map.T.reshape(8, P, K).transpose(1, 0, 2)).astype(BF16)
    b_att_c = np.ascontiguousarray(b_att[:, None])
    b_map_c = np.ascontiguousarray(b_map[:, None])

    shared = dict(tab_lo=tab_lo, tab_hi=tab_hi, w_attT=w_attT, w_mapT=w_mapT,
                  b_att=b_att_c, b_map=b_map_c)

    in_maps = []
    for c in range(NCORES):
        xc = x[c * BC:(c + 1) * BC]
        hist = xc[:, 4:4 + M].astype(np.int64)           # (2048, 50)
        # m-major per tile: c = m*128 + b
        idx = hist.reshape(NT, P, M).transpose(0, 2, 1).reshape(NT, CPT)
        lo = np.where(idx < SPLIT, idx, 0).astype(np.int16)
        hi = np.where(idx >= SPLIT, idx - SPLIT, NHI - 1).astype(np.int16)
        # wrap in 16 partitions: pos i -> [i % 16, i // 16]
        lo_w = lo.reshape(NT, CPT // 16, 16).transpose(0, 2, 1)   # (NT, 16, 400)
        hi_w = hi.reshape(NT, CPT // 16, 16).transpose(0, 2, 1)
        lo_w = np.tile(lo_w, (1, 8, 1)).reshape(NT * P, CPT // 16)
        hi_w = np.tile(hi_w, (1, 8, 1)).reshape(NT * P, CPT // 16)

        uT = userVecs[xc[:, 0]].T.reshape(2, P, BC).transpose(1, 0, 2)
        in_maps.append(dict(
            shared,
            idx_lo=np.ascontiguousarray(lo_w), idx_hi=np.ascontiguousarray(hi_w),
            uT_bf=np.ascontiguousarray(uT.astype(BF16)),
            auxT=np.ascontiguousarray(np.stack([
                tagU[xc[:, 2]].T, tagU[xc[:, 3]].T,
                tagI[xc[:, 2]].T, tagI[xc[:, 3]].T,
                itemVecs[xc[:, 1]].T], axis=1).reshape(2, P, 5, BC).transpose(1, 0, 2, 3)),
        ))
    return in_maps


def kernel(**inputs):
    if "nc" not in _nc_cache:
        _nc_cache["nc"] = _build_program()
    nc = _nc_cache["nc"]
    in_maps = _host_prep(inputs)
    res = run_bass_kernel_spmd(nc, in_maps, list(range(NCORES)))
    _nc_cache["last_res"] = res
    outs = [res.results[c]["r_out"].reshape(BC) for c in range(NCORES)]
    r = np.concatenate(outs, 0).astype(np.float32)
    return r[:, None, None]

